# revision 35
# baseline (speedup 1.0000x reference)
"""Bass/Trainium2 kernel for a 2-layer GAT (nn_GAT_48919677501958).

Contract: kernel(**inputs) takes the FULL unsharded numpy inputs (keyed as in
setup_inputs()) and returns the FULL [10000, 40] float32 output.

Strategy (8 NeuronCores, SPMD single program), v2 "edge-major + PE aggregation":
  - Host: append self-loops, assign nodes to 80 global dst-tiles of 128
    (degree-snake for edge balance), snake tiles onto 8 cores x 10 ranks.
    Per tile, its incoming edges (sorted by dst slot) are laid out
    edge-major: edge position i -> (chunk i//128, partition i%128).
  - Device per core:
      Phase A: own-shard H = X@W1 (+ [alpha_src | alpha_dst] halves) fp32r
               matmuls; rows [h(512, head-interleaved) | a_src(8) | pad] in
               bf16 -> table `haug`; AllGather (the only L1 collective).
      Phase S: one-hot matrices S[e,d] = (dstid[e] == d) built on Pool/DVE
               from a replicated iota row; S^T per tile from a broadcast
               dstid; both bf16.
      Phase B per tile: dma_gather edge rows (edges on partitions);
               alpha_dst per edge = S^T-chunk @ ad_tile (PE, trivial);
               logits + LeakyReLU (DVE decomp) + Exp (ACT) -> ex overwrites
               the a_src columns; DVE scales h rows by ex (the only big DVE
               op); segment-sum + softmax denominator = per-chunk PE matmuls
               S^T.T @ [ex*h | ex] accumulated in PSUM. Normalize + ELU at
               node level; layer-2 transform (PE transposes + matmul) ->
               rows [h2(40) | a_src2(1)] bf16 -> h2own; alpha_dst2 per edge
               precomputed while S^T is live.
      AllGather h2own -> h2all.
      Phase D per tile: 256B-row gather by the same indices; same scheme
               41-wide; normalize -> out rows.
  - Host: concat per-core outputs, inverse-permute rows.
"""

from dataclasses import dataclass, field

import numpy as np

import concourse.bass as bass
import concourse.mybir as mybir
import concourse.tile as tile
from concourse.bass_utils import run_bass_kernel_spmd
from concourse.masks import make_identity

F32 = mybir.dt.float32
F32R = mybir.dt.float32r
BF16 = mybir.dt.bfloat16
I16 = mybir.dt.int16

NEG_SLOPE = 0.2
P = 128
PAD_D = 999.0  # dstid sentinel for pad edges: matches no one-hot column


@dataclass
class Cfg:
    n_nodes: int  # real node count (10000)
    n_cores: int  # 8
    tpc: int  # tiles per core (10)
    d_in: int  # 256
    hid: int  # 64
    heads: int  # 8
    d_out: int  # 40
    c_prog: list[int] = field(default_factory=list)  # chunks per tile rank
    have_b1: bool = False
    have_b2: bool = False
    s_pool: int = 5  # chunks per tile whose S-build runs on Pool (rest DVE)
    collective: bool = True  # False: replace AllGather with local copy (model)
    phases: str = "ABCD"  # phase prefix to emit (model/debug)

    @property
    def npc(self):  # padded nodes per core
        return self.tpc * P

    @property
    def npad(self):
        return self.n_cores * self.npc

    @property
    def n_tiles(self):
        return self.n_cores * self.tpc

    @property
    def d_hid(self):  # concat hidden width (512)
        return self.hid * self.heads

    @property
    def rw1(self):  # haug row width in bf16 elements (row bytes % 256)
        w = self.d_hid + self.heads
        return (w + 127) // 128 * 128

    @property
    def rw2(self):  # h2all row width in bf16 elements (256B)
        return P

    @property
    def sum_c(self):
        return sum(self.c_prog)


def _wrap_idx(flat: np.ndarray) -> np.ndarray:
    """dma_gather index layout: position i lives at [i % 16, i // 16],
    replicated across the 8 GpSimd-core stripes of 16 partitions each."""
    assert flat.size % 16 == 0
    w = np.ascontiguousarray(flat.reshape(-1, 16).T).astype(np.int16)
    return np.tile(w, (8, 1))


def preprocess(cfg: Cfg, x, edge_index, W1, att_src1, att_dst1, b1, W2,
               att_src2, att_dst2, b2):
    """Host-side graph/layout preprocessing. Returns (in_maps, node_of_slot)."""
    N = cfg.n_nodes
    NT = cfg.n_tiles
    src = np.concatenate([np.asarray(edge_index[0], np.int64), np.arange(N)])
    dst = np.concatenate([np.asarray(edge_index[1], np.int64), np.arange(N)])
    deg = np.bincount(dst, minlength=N)

    # CSR by dst
    order_e = np.argsort(dst, kind="stable")
    sorted_src = src[order_e]
    starts = np.zeros(N + 1, np.int64)
    np.cumsum(deg, out=starts[1:])

    # degree-sorted nodes, snake round-robin into NT global tiles of 128
    node_order = np.argsort(-deg, kind="stable")
    padded = np.full(cfg.npad, -1, np.int64)
    padded[:N] = node_order
    tiles = np.full((NT, P), -1, np.int64)
    for r in range(P):
        row = padded[r * NT:(r + 1) * NT]
        if r % 2:
            row = row[::-1]
        tiles[:, r] = row
    tile_e = np.where(tiles >= 0, deg[np.maximum(tiles, 0)], 0).sum(axis=1)

    # tiles sorted by edge count desc; rank r <- tiles [r*8, (r+1)*8)
    t_order = np.argsort(-tile_e, kind="stable")
    core_tiles = [[int(t_order[r * cfg.n_cores + c]) for r in range(cfg.tpc)]
                  for c in range(cfg.n_cores)]
    cfg.c_prog = [
        int(max((tile_e[core_tiles[c][r]] + P - 1) // P
                for c in range(cfg.n_cores)))
        for r in range(cfg.tpc)
    ]

    # slot maps
    node_of_slot = np.full((cfg.n_cores, cfg.npc), -1, np.int64)
    for c in range(cfg.n_cores):
        for r in range(cfg.tpc):
            node_of_slot[c, r * P:(r + 1) * P] = tiles[core_tiles[c][r]]
    row_of_node = np.full(N, -1, np.int64)
    flat_slots = node_of_slot.reshape(-1)
    real = flat_slots >= 0
    row_of_node[flat_slots[real]] = np.nonzero(real)[0]
    assert (row_of_node >= 0).all()

    # permuted, padded, transposed x
    xT = np.zeros((cfg.d_in, cfg.npad), np.float32)
    xT[:, np.nonzero(real)[0]] = np.asarray(x, np.float32).T[:, flat_slots[real]]

    # packed weights (host weight-folding only)
    W1 = np.asarray(W1, np.float32)
    ablk_s = np.zeros((cfg.d_hid, cfg.heads), np.float32)
    ablk_d = np.zeros((cfg.d_hid, cfg.heads), np.float32)
    a_s1 = np.asarray(att_src1, np.float32)
    a_d1 = np.asarray(att_dst1, np.float32)
    for h in range(cfg.heads):
        ablk_s[h * cfg.hid:(h + 1) * cfg.hid, h] = a_s1[h]
        ablk_d[h * cfg.hid:(h + 1) * cfg.hid, h] = a_d1[h]
    Wa1 = np.concatenate([W1 @ ablk_s, W1 @ ablk_d], axis=1)  # [d_in, 2*heads]
    W2 = np.asarray(W2, np.float32)
    w2s = W2 @ np.asarray(att_src2, np.float32)[0]
    w2d = W2 @ np.asarray(att_dst2, np.float32)[0]
    W2a = np.concatenate([W2, w2s[:, None], w2d[:, None]], axis=1)  # [512, 42]
    b1 = np.asarray(b1, np.float32)
    b2 = np.asarray(b2, np.float32)
    cfg.have_b1 = bool(np.abs(b1).max() > 0)
    cfg.have_b2 = bool(np.abs(b2).max() > 0)
    # interleaved hidden layout: new col j=(c,h) maps to old col h*hid+c
    j = np.arange(cfg.d_hid)
    old = (j % cfg.heads) * cfg.hid + j // cfg.heads
    b1r = np.tile(b1[None, old], (P, 1))
    b2r = np.tile(b2[None, :], (P, 1))
    W2a = np.ascontiguousarray(W2a[old, :])

    iota_rep = np.tile(np.arange(P, dtype=np.float32)[None, :], (P, 1))
    iota_pp = np.arange(P, dtype=np.float32)[:, None]

    # per-core edge-major layouts
    in_maps = []
    for c in range(cfg.n_cores):
        gi_parts, dpp_parts = [], []
        for r in range(cfg.tpc):
            C = cfg.c_prog[r]
            nodes = node_of_slot[c, r * P:(r + 1) * P]
            srcs, dids = [], []
            for d in range(P):
                n = nodes[d]
                if n >= 0:
                    k = int(deg[n])
                    srcs.append(row_of_node[sorted_src[starts[n]:starts[n] + k]])
                    dids.append(np.full(k, d, np.float32))
            srcs = np.concatenate(srcs) if srcs else np.zeros(0, np.int64)
            dids = np.concatenate(dids) if dids else np.zeros(0, np.float32)
            npos = C * P
            assert srcs.size <= npos
            gi_t = np.zeros(npos, np.int64)
            gi_t[:srcs.size] = srcs
            dpp_t = np.full(npos, PAD_D, np.float32)
            dpp_t[:dids.size] = dids
            gi_parts.append(_wrap_idx(gi_t))
            dpp_parts.append(dpp_t.reshape(C, P).T)  # [128, C]
        gi = np.concatenate(gi_parts, axis=1)
        dpp = np.ascontiguousarray(np.concatenate(dpp_parts, axis=1))
        in_maps.append({
            "xTo": np.ascontiguousarray(xT[:, c * cfg.npc:(c + 1) * cfg.npc]),
            "W1": W1, "Wa1": Wa1, "W2a": W2a, "b1r": b1r, "b2r": b2r,
            "gi": gi, "dpp": dpp,
            "iotar": iota_rep,
        })
    return in_maps, node_of_slot


def build_program(cfg: Cfg) -> bass.Bass:
    import concourse.bacc as bacc
    nc = bacc.Bacc("TRN2", target_bir_lowering=False, num_devices=cfg.n_cores)
    DH, HD, DO = cfg.d_hid, cfg.heads, cfg.d_out
    KT = cfg.d_in // P  # k-tiles for layer-1 matmul
    K2 = DH // P        # k-tiles for layer-2 matmul
    SC = cfg.sum_c      # total chunks per core
    Cmax = max(cfg.c_prog)
    doB = "B" in cfg.phases
    doC = "C" in cfg.phases
    doD = "D" in cfg.phases

    # ---- DRAM ----
    xTo = nc.dram_tensor("xTo", [cfg.d_in, cfg.npc], F32R,
                         kind="ExternalInput")
    W1 = nc.dram_tensor("W1", [cfg.d_in, DH], F32R, kind="ExternalInput")
    Wa1 = nc.dram_tensor("Wa1", [cfg.d_in, 2 * HD], F32R,
                        kind="ExternalInput")
    W2a = nc.dram_tensor("W2a", [DH, DO + 2], F32, kind="ExternalInput")
    b1r = nc.dram_tensor("b1r", [P, DH], F32, kind="ExternalInput")
    b2r = nc.dram_tensor("b2r", [P, DO], F32, kind="ExternalInput")
    gi = nc.dram_tensor("gi", [P, SC * 8], I16, kind="ExternalInput")
    dpp = nc.dram_tensor("dpp", [P, SC], F32, kind="ExternalInput")
    iotar = nc.dram_tensor("iotar", [P, P], F32, kind="ExternalInput")
    out = nc.dram_tensor("out", [cfg.npc, DO], F32, kind="ExternalOutput")

    haug = nc.dram_tensor("haug", [cfg.npad, cfg.rw1], BF16,
                          addr_space="Shared" if cfg.collective else "Local")
    haug_own = nc.dram_tensor("haug_own", [cfg.npc, cfg.rw1], BF16)
    h2own = nc.dram_tensor("h2own", [cfg.npc, cfg.rw2], BF16)
    h2all = nc.dram_tensor("h2all", [cfg.npad, cfg.rw2], BF16,
                           addr_space="Shared" if cfg.collective else "Local")

    from contextlib import ExitStack
    with tile.TileContext(nc) as tc, ExitStack() as st:
        cst = st.enter_context(tc.tile_pool(name="cst", bufs=1))
        hsb_p = st.enter_context(tc.tile_pool(name="hsb", bufs=5))
        acc_p = st.enter_context(tc.tile_pool(name="acc", bufs=2, space="PSUM"))
        aux_p = st.enter_context(tc.tile_pool(name="aux", bufs=2, space="PSUM"))
        psT_p = st.enter_context(tc.tile_pool(name="psT", bufs=2, space="PSUM"))
        ps2_p = st.enter_context(tc.tile_pool(name="ps2", bufs=2, space="PSUM"))
        hg_p = st.enter_context(tc.tile_pool(name="hg", bufs=3))

        sm_p = st.enter_context(tc.tile_pool(name="sm", bufs=4))
        big_p = st.enter_context(tc.tile_pool(name="big", bufs=2))
        hg2_p = st.enter_context(tc.tile_pool(name="hg2", bufs=3))
        out_p = st.enter_context(tc.tile_pool(name="outp", bufs=4))

        # ---- constants to SBUF ----
        w1sb = cst.tile([P, KT, DH], F32R)
        wa1sb = cst.tile([P, KT, 2 * HD], F32R)
        w2sb = cst.tile([P, K2, DO + 2], BF16)
        xosb = cst.tile([P, KT, cfg.npc], F32R)
        gisb = cst.tile([P, SC * 8], I16)
        dppsb = cst.tile([P, SC], F32)
        iotarsb = cst.tile([P, P], BF16)
        ident = cst.tile([P, P], BF16)
        ad1_sb = cst.tile([P, cfg.tpc * HD], BF16)
        ad2_sb = cst.tile([P, cfg.tpc], BF16)
        ad2e_sb = cst.tile([P, SC], F32)
        s_all = cst.tile([P, SC, P], BF16)
        stt_all = cst.tile([P, SC, P], BF16)
        b1sb = cst.tile([P, DH], F32) if cfg.have_b1 else None
        b2sb = cst.tile([P, DO], F32) if cfg.have_b2 else None

        for k in range(KT):
            nc.sync.dma_start(out=w1sb[:, k, :], in_=W1[k * P:(k + 1) * P, :])
            nc.sync.dma_start(out=wa1sb[:, k, :],
                              in_=Wa1[k * P:(k + 1) * P, :])
            nc.sync.dma_start(out=xosb[:, k, :],
                              in_=xTo[k * P:(k + 1) * P, :])
        for k in range(K2):
            nc.gpsimd.dma_start(out=w2sb[:, k, :], in_=W2a[k * P:(k + 1) * P, :])
        nc.sync.dma_start(out=gisb[:], in_=gi[:])
        nc.sync.dma_start(out=dppsb[:], in_=dpp[:])
        nc.gpsimd.dma_start(out=iotarsb[:], in_=iotar[:])
        if cfg.have_b1:
            nc.sync.dma_start(out=b1sb[:], in_=b1r[:])
        if cfg.have_b2:
            nc.sync.dma_start(out=b2sb[:], in_=b2r[:])
        make_identity(nc, ident[:])

        # ---- emission helpers (software-pipelined across tiles) ----
        toff = [0]
        for r in range(cfg.tpc):
            toff.append(toff[r] + cfg.c_prog[r])

        def s_build(t):
            # one-hot S chunks for tile t (dep: iotar + dpp consts only)
            for c in range(cfg.c_prog[t]):
                cc = toff[t] + c
                eng = nc.gpsimd if c < cfg.s_pool else nc.vector
                eng.tensor_scalar(
                    out=s_all[:, cc, :], in0=iotarsb[:],
                    scalar1=dppsb[:, cc:cc + 1], scalar2=None,
                    op0=mybir.AluOpType.is_equal)

        def phase_a(t):
            lt = xosb[:, :, t * P:(t + 1) * P]
            php = acc_p if t % 2 == 0 else ps2_p
            ph = php.tile([P, DH], F32, tag="acc" if t % 2 == 0 else "ps2")
            pa_t = aux_p.tile([P, DH], F32, tag="aux")
            pa = pa_t[:, :2 * HD]
            for k in range(KT):
                nc.tensor.matmul(ph[:], lt[:, k, :], w1sb[:, k, :],
                                 start=(k == 0), stop=(k == KT - 1))
            for k in range(KT):
                nc.tensor.matmul(pa[:], lt[:, k, :], wa1sb[:, k, :],
                                 start=(k == 0), stop=(k == KT - 1))
            hs = hsb_p.tile([P, cfg.rw1], BF16, tag="hsb")
            # head-interleaved: col j=(c,h); psum is (h,c)
            nc.vector.tensor_copy(
                hs[:, :DH].rearrange("p (c h) -> p h c", h=HD),
                ph[:].rearrange("p (h c) -> p h c", h=HD))
            nc.vector.tensor_copy(hs[:, DH:DH + HD], pa[:, :HD])
            nc.vector.tensor_copy(ad1_sb[:, t * HD:(t + 1) * HD],
                                  pa[:, HD:2 * HD])
            dst = haug_own if cfg.collective else haug
            nc.sync.dma_start(
                out=dst[t * P:(t + 1) * P, :DH + HD],
                in_=hs[:, :DH + HD])

        def stt_prep(t):
            """PE-transpose tile t's S chunks into the persistent S^T buffer."""
            C = cfg.c_prog[t]
            g_off = toff[t]
            for b in range((C + 7) // 8):
                cb = min(8, C - b * 8)
                ptt = psT_p.tile([P, 8, P], BF16, tag="psT")
                for j in range(cb):
                    nc.tensor.transpose(ptt[:, j, :],
                                        s_all[:, g_off + b * 8 + j, :],
                                        ident[:])
                nc.scalar.copy(stt_all[:, g_off + b * 8:g_off + b * 8 + cb, :],
                               ptt[:, :cb, :])

        def prep_b(t):
            """Gather tile t + alpha_dst matmuls."""
            C = cfg.c_prog[t]
            g_off = toff[t]
            nidx = C * P
            hg = hg_p.tile([P, Cmax, cfg.rw1], BF16, tag="hg")
            hgv = hg[:, :C, :]
            nc.gpsimd.dma_gather(
                out_ap=hgv, in_ap=haug[:, :],
                idxs_ap=gisb[:, g_off * 8:(g_off + C) * 8],
                num_idxs=nidx, num_idxs_reg=nidx, elem_size=cfg.rw1,
                single_packet=False)
            # alpha_dst per edge: S^T-chunk @ ad_tile
            aux_t = aux_p.tile([P, DH], F32, tag="aux")
            psAd = aux_t[:, :C * HD].rearrange("p (c h) -> p c h", h=HD)
            for c in range(C):
                nc.tensor.matmul(psAd[:, c, :], stt_all[:, g_off + c, :],
                                 ad1_sb[:, t * HD:(t + 1) * HD],
                                 start=True, stop=True)
            return hgv, None, aux_t

        def compute_edge(t, hgv, stt, aux_t):
            """Logits, ex, scale, den+msg matmuls for tile t (two halves)."""
            C = cfg.c_prog[t]
            g_off = toff[t]
            psAd = aux_t[:, :C * HD].rearrange("p (c h) -> p c h", h=HD)
            p2_t = ps2_p.tile([P, DH], F32, tag="ps2")
            psD = p2_t[:, 128:128 + HD]
            psM = acc_p.tile([P, DH], F32, tag="acc")
            lg = sm_p.tile([P, Cmax, HD], F32, tag="lg")
            neg = sm_p.tile([P, Cmax, HD], F32, tag="neg")
            h0 = (C + 2) // 3
            parts = [(0, min(h0, C)), (min(h0, C), min(2 * h0, C)),
                     (min(2 * h0, C), C)]
            for lo, hi in [(a, b) for a, b in parts if b > a]:
                n = hi - lo
                lgv = lg[:, lo:hi, :]
                negv = neg[:, lo:hi, :]
                exv = hgv[:, lo:hi, DH:DH + HD]
                # logits: as (gathered cols) + ad; LeakyReLU via DVE decomp
                nc.vector.tensor_tensor(out=lgv, in0=exv,
                                        in1=psAd[:, lo:hi, :],
                                        op=mybir.AluOpType.add)
                nc.vector.tensor_scalar_min(out=negv, in0=lgv, scalar1=0.0)
                nc.vector.tensor_scalar_max(out=lgv, in0=lgv, scalar1=0.0)
                nc.vector.scalar_tensor_tensor(
                    out=lgv, in0=negv, scalar=NEG_SLOPE, in1=lgv,
                    op0=mybir.AluOpType.mult, op1=mybir.AluOpType.add)
                # ex = exp(logit) -> overwrite the a_src columns (bf16)
                nc.scalar.activation(exv, lgv,
                                     mybir.ActivationFunctionType.Exp)
                # scale rows by ex (head-interleaved -> unit stride, DVE 2x)
                hgm = hgv[:, lo:hi, :DH].rearrange("p s (c h) -> p s c h",
                                                   h=HD)
                nc.vector.tensor_tensor(
                    out=hgm, in0=hgm,
                    in1=exv.unsqueeze(2).broadcast_to([P, n, cfg.hid, HD]),
                    op=mybir.AluOpType.mult)
                # denominator + weighted-message matmuls
                for c in range(lo, hi):
                    nc.tensor.matmul(psD[:], s_all[:, g_off + c, :],
                                     hgv[:, c, DH:DH + HD],
                                     start=(c == 0), stop=(c == C - 1))
                for c in range(lo, hi):
                    nc.tensor.matmul(psM[:], s_all[:, g_off + c, :],
                                     hgv[:, c, :DH],
                                     start=(c == 0), stop=(c == C - 1))
            return psM, p2_t

        def compute_node(t, stt, psM, p2_t):
            """Normalize + ELU + layer-2 transform for tile t."""
            C = cfg.c_prog[t]
            g_off = toff[t]
            psD = p2_t[:, 128:128 + HD]
            den = sm_p.tile([P, HD], F32, tag="den")
            nc.vector.tensor_scalar_add(out=den[:], in0=psD, scalar1=1e-30)
            rec = sm_p.tile([P, HD], F32, tag="rec")
            nc.vector.reciprocal(rec[:], den[:])
            v = big_p.tile([P, DH], F32, tag="v")
            nc.vector.tensor_tensor(
                out=v[:].rearrange("p (c h) -> p c h", h=HD),
                in0=psM[:].rearrange("p (c h) -> p c h", h=HD),
                in1=rec[:].unsqueeze(1).broadcast_to([P, cfg.hid, HD]),
                op=mybir.AluOpType.mult)
            if cfg.have_b1:
                nc.vector.tensor_add(v[:], v[:], b1sb[:])
            y = big_p.tile([P, DH], BF16, tag="y")
            tneg = big_p.tile([P, DH], F32, tag="tneg")
            nc.scalar.activation(tneg[:], v[:],
                                 mybir.ActivationFunctionType.Relu, scale=-1.0)
            nc.scalar.activation(v[:], v[:],
                                 mybir.ActivationFunctionType.Relu)
            nc.scalar.activation(tneg[:], tneg[:],
                                 mybir.ActivationFunctionType.Exp, scale=-1.0)
            nc.vector.scalar_tensor_tensor(
                out=y[:], in0=tneg[:], scalar=-1.0, in1=v[:],
                op0=mybir.AluOpType.add, op1=mybir.AluOpType.add)
            if not doC:
                return
            # ---- Phase C: transpose y, layer-2 matmul ----
            yT = big_p.tile([P, K2, P], BF16, tag="yT")
            pt = psT_p.tile([P, 8, P], BF16, tag="psT")
            for k in range(K2):
                nc.tensor.transpose(pt[:, k, :], y[:, k * P:(k + 1) * P],
                                    ident[:])
            nc.scalar.copy(yT[:], pt[:, :K2, :])
            p2 = p2_t[:, :DO + 2]
            for k in range(K2):
                nc.tensor.matmul(p2[:], yT[:, k, :], w2sb[:, k, :],
                                 start=(k == 0), stop=(k == K2 - 1))
            h2sb = out_p.tile([P, DO + 1], BF16, tag="h2sb")
            nc.scalar.copy(h2sb[:], p2[:, :DO + 1])
            nc.scalar.copy(ad2_sb[:, t:t + 1], p2[:, DO + 1:DO + 2])
            dst2 = h2own if cfg.collective else h2all
            nc.sync.dma_start(out=dst2[t * P:(t + 1) * P, :DO + 1],
                              in_=h2sb[:])
            # alpha_dst2 per edge while S^T is live
            psA2 = p2_t[:, 64:64 + Cmax]
            for c in range(C):
                nc.tensor.matmul(psA2[:, c:c + 1], stt_all[:, g_off + c, :],
                                 ad2_sb[:, t:t + 1], start=True, stop=True)
            nc.vector.tensor_copy(ad2e_sb[:, g_off:g_off + C], psA2[:, :C])

        def prep_d(t):
            C = cfg.c_prog[t]
            g_off = toff[t]
            nidx = C * P
            hg2 = hg2_p.tile([P, Cmax, cfg.rw2], BF16, tag="hg2")
            hg2v = hg2[:, :C, :]
            nc.gpsimd.dma_gather(
                out_ap=hg2v, in_ap=h2all[:, :],
                idxs_ap=gisb[:, g_off * 8:(g_off + C) * 8],
                num_idxs=nidx, num_idxs_reg=nidx, elem_size=cfg.rw2,
                single_packet=False)
            return hg2v

        def compute_d_edge(t, hg2v):
            C = cfg.c_prog[t]
            g_off = toff[t]
            lg2 = sm_p.tile([P, Cmax], F32, tag="lg2")
            lg2v = lg2[:, :C]
            nc.vector.tensor_tensor(
                out=lg2v, in0=hg2v[:, :, DO:DO + 1].squeeze(),
                in1=ad2e_sb[:, g_off:g_off + C], op=mybir.AluOpType.add)
            neg2 = sm_p.tile([P, Cmax], F32, tag="neg2")
            neg2v = neg2[:, :C]
            nc.vector.tensor_scalar_min(out=neg2v, in0=lg2v, scalar1=0.0)
            nc.vector.tensor_scalar_max(out=lg2v, in0=lg2v, scalar1=0.0)
            nc.vector.scalar_tensor_tensor(
                out=lg2v, in0=neg2v, scalar=NEG_SLOPE, in1=lg2v,
                op0=mybir.AluOpType.mult, op1=mybir.AluOpType.add)
            nc.scalar.activation(hg2v[:, :, DO:DO + 1].squeeze(), lg2v,
                                 mybir.ActivationFunctionType.Exp)
            nc.vector.tensor_tensor(
                out=hg2v[:, :, :DO], in0=hg2v[:, :, :DO],
                in1=hg2v[:, :, DO:DO + 1].broadcast_to([P, C, DO]),
                op=mybir.AluOpType.mult)
            psM2_t = ps2_p.tile([P, DH], F32, tag="ps2")
            psM2 = psM2_t[:, :DO + 1]
            for c in range(C):
                nc.tensor.matmul(psM2[:], s_all[:, g_off + c, :],
                                 hg2v[:, c, :DO + 1],
                                 start=(c == 0), stop=(c == C - 1))
            return psM2_t

        def compute_d_node(t, psM2_t):
            psM2 = psM2_t[:, :DO + 1]
            den2 = sm_p.tile([P, 1], F32, tag="den2")
            nc.vector.tensor_scalar_add(out=den2[:], in0=psM2[:, DO:DO + 1],
                                        scalar1=1e-30)
            rec2 = sm_p.tile([P, 1], F32, tag="rec2")
            nc.vector.reciprocal(rec2[:], den2[:])
            osb = out_p.tile([P, DO], F32, tag="osb")
            nc.vector.tensor_scalar(out=osb[:], in0=psM2[:, :DO],
                                    scalar1=rec2[:, :1], scalar2=None,
                                    op0=mybir.AluOpType.mult)
            if cfg.have_b2:
                nc.vector.tensor_add(osb[:], osb[:], b2sb[:])
            nc.sync.dma_start(out=out[t * P:(t + 1) * P, :], in_=osb[:])

        # ---- emission: A, then pipelined S/B/C, AG2, pipelined D ----
        if doB:
            for t in range(min(3, cfg.tpc)):
                s_build(t)
        for t in range(cfg.tpc):
            phase_a(t)
        if doB:
            stt_prep(0)
            stt_prep(1)

        if cfg.collective:
            nc.gpsimd.collective_compute(
                "AllGather", mybir.AluOpType.bypass,
                ins=[haug_own[:]], outs=[haug[:]],
                replica_groups=[list(range(cfg.n_cores))])

        if doB:
            st1 = st2 = None
            for t in range(cfg.tpc):
                cur = (t, *prep_b(t))
                if t + 3 < cfg.tpc:
                    s_build(t + 3)
                if t + 2 < cfg.tpc:
                    stt_prep(t + 2)
                if st1 is not None:
                    t1, hgv1, stt1, aux1 = st1
                    e1 = (t1, stt1, *compute_edge(t1, hgv1, stt1, aux1))
                    if st2 is not None:
                        compute_node(*st2)
                    st2 = e1
                st1 = cur
            t1, hgv1, stt1, aux1 = st1
            e1 = (t1, stt1, *compute_edge(t1, hgv1, stt1, aux1))
            compute_node(*st2)
            compute_node(*e1)

        if doB and doC and doD:
            if cfg.collective:
                nc.gpsimd.collective_compute(
                    "AllGather", mybir.AluOpType.bypass,
                    ins=[h2own[:]], outs=[h2all[:]],
                    replica_groups=[list(range(cfg.n_cores))])
            st1 = st2 = None
            for t in range(cfg.tpc):
                cur = (t, prep_d(t))
                if st1 is not None:
                    t1, hg2v1 = st1
                    e1 = (t1, compute_d_edge(t1, hg2v1))
                    if st2 is not None:
                        compute_d_node(*st2)
                    st2 = e1
                st1 = cur
            t1, hg2v1 = st1
            e1 = (t1, compute_d_edge(t1, hg2v1))
            compute_d_node(*st2)
            compute_d_node(*e1)

    nc.compile()
    return nc


def default_cfg() -> Cfg:
    return Cfg(n_nodes=10000, n_cores=8, tpc=10, d_in=256, hid=64, heads=8,
               d_out=40)


def run(inputs: dict, cfg: Cfg | None = None, **run_kwargs):
    cfg = cfg or default_cfg()
    in_maps, node_of_slot = preprocess(cfg, **inputs)
    nc = build_program(cfg)
    res = run_bass_kernel_spmd(nc, in_maps, list(range(cfg.n_cores)),
                               **run_kwargs)
    outs = np.concatenate([res.results[c]["out"] for c in range(cfg.n_cores)],
                          axis=0)
    full = np.zeros((cfg.n_nodes, cfg.d_out), np.float32)
    flat = node_of_slot.reshape(-1)
    real = flat >= 0
    full[flat[real]] = outs[real]
    return full, res


def kernel(**inputs) -> np.ndarray:
    out, _ = run(inputs)
    return out
